# revision 25
# baseline (speedup 1.0000x reference)
"""Trainium2 Bass kernel for nn_BasisFunction2D (2-D basis-function embedding lookup).

Reformulation: data-dependent bilinear interpolation over a 16x16 grid of
per-(ix,iz) tables expressed as dense hat-function interpolation matrices

    V[(q,iz), b] = hat_q(z[iz,b])      (z-side weights, 2 nonzeros per column)
    U[(p,ix), b] = hat_p(x[ix,b])      (x-side weights)

with partition-of-unity folds on BOTH axes (sum_q hat_q = sum_p hat_p = 1,
exact even in the linear-extrapolation tails), which shrink the contraction
to K=512 (4 full PE chunks, no ragged tail) and the free dim to M=512 (one
PSUM bank per chain):

    out[o,b] = sum_m C_o[b,m] * U[m,b]  +  side[o,b]
    C_o[b,m] = sum_{k in 512} V[k,b] * Ghh_o[k,m]         (PE, bf16, N=512)
    Ghh = doubly-folded table;  side = cbt_o @ U + gr2_o @ V + c0_o is a
    rank-small exact correction (0.35% of the FLOPs) evaluated on the host
    in fp32 and added during output assembly.

Within an output column o the four batch chains are emitted kc-major
(interleaved), so consecutive matmuls target different PSUM banks and
pipeline through the PE reorder window at the N=512 streaming rate.

The hat arms L/R are host-precomputed affine maps of z/x (bf16); the device
builds V = relu(min(L,R)) on the vector engine (2 ops/chunk + 1-op stt fixes
for the extrapolation rows).  Stage 2 drains each chain with an ACT
PSUM->SBUF bf16 copy + DVE fused multiply-reduce against U (the last column
reduces straight out of PSUM to shorten the tail).
"""

import numpy as np

import concourse.bass as bass
import concourse.bacc as bacc_mod
import concourse.tile as tile
from concourse import mybir
from concourse.bass_utils import run_bass_kernel_spmd

F32 = mybir.dt.float32
BF16 = mybir.dt.bfloat16
AF = mybir.ActivationFunctionType
ALU = mybir.AluOpType

NCORES = 8
NG = 16            # grid bins
NQ = 17            # grid corners per axis
IX = 32
IZ = 32
OUT = 64
B = 512
OSH = OUT // NCORES          # outputs per core = 8
KF = NG * IZ                 # 512 folded contraction rows (q<=15, iz)
M = NG * IX                  # 512 folded free cols (p<=15, ix)
BIG = 1e30
NBC = B // 128               # 4 batch chunks
NKC = 4                      # contraction chunks of 128
NWARM = 10                   # PE warmup matmuls (p-state ramp + DMA cover)

_NC_CACHE = {}


def _build_nc(bd, il):
    """Build the single-core Bass/Tile program (identical across cores)."""
    nc = bacc_mod.Bacc(None, target_bir_lowering=False)
    gmain_d = nc.dram_tensor("gmain", [OSH, 128, NKC * M], BF16, kind="ExternalInput")
    zvl_d = nc.dram_tensor("zvl", [128, NKC * B], BF16, kind="ExternalInput")
    zvr_d = nc.dram_tensor("zvr", [128, NKC * B], BF16, kind="ExternalInput")
    ulr_d = nc.dram_tensor("ulr", [128, 8 * M], BF16, kind="ExternalInput")
    out_d = nc.dram_tensor("out", [B, OSH], F32, kind="ExternalOutput")

    with tile.TileContext(nc) as tc:
        with (
            tc.tile_pool(name="per", bufs=1) as per,       # persistent tiles
            tc.tile_pool(name="scb", bufs=4) as scb,       # stage2 ACT copies
            tc.tile_pool(name="junk", bufs=2) as junk,     # stt mandatory outs
            tc.tile_pool(name="ps", bufs=7, space="PSUM") as ps,
            tc.tile_pool(name="ps2", bufs=1, space="PSUM") as ps2,
        ):
            # ---------------- PE warmup ----------------
            # Dependency-free dummy matmuls keep the PE busy through the DMA
            # and V build phase so the p-state ramps before the real chains.
            wt = per.tile([128, B], BF16, tag="warm", name="wt")
            nc.vector.memset(wt[:], 0.0)
            wps = ps2.tile([128, B], F32, tag="w", name="wps")
            for _ in range(NWARM):
                nc.tensor.matmul(wps[:], wt[:, 0:128], wt[:], start=True, stop=True)

            # ---------------- input loads ----------------
            # V arm chunks split across the sync and scalar trigger queues
            # (each trigger costs ~650ns of queue time); the gpsimd queue
            # (slow: framework drains) gets the early o=0 G tile.
            zvLt = per.tile([128, NKC * B], BF16, tag="zvL", name="zvLt")
            zvRt = per.tile([128, NKC * B], BF16, tag="zvR", name="zvRt")
            uLRt = per.tile([128, 8 * M], BF16, tag="uLR", name="uLRt")
            G_sb = [per.tile([128, NKC * M], BF16, tag=f"G{o}", name=f"G{o}")
                    for o in range(OSH)]

            # G0 + the V arms gate the first main chains: G0 fires first on
            # sync, arms split across sync/scalar, the rest trails.  uLR goes
            # to the slow gpsimd queue (needed latest).
            nc.sync.dma_start(G_sb[0][:], gmain_d[0])
            for kci in range(NKC):
                cs = slice(kci * B, (kci + 1) * B)
                nc.sync.dma_start(zvLt[:, cs], zvl_d[:, cs])
                nc.scalar.dma_start(zvRt[:, cs], zvr_d[:, cs])
            for bc in range(NBC):
                ub = slice(bc * 2 * M, (bc + 1) * 2 * M)
                nc.gpsimd.dma_start(uLRt[:, ub], ulr_d[:, ub])
            for o in range(1, 4):
                nc.sync.dma_start(G_sb[o][:], gmain_d[o])
            for o in range(4, OSH):
                nc.scalar.dma_start(G_sb[o][:], gmain_d[o])

            outT_sb = [per.tile([128, OSH], F32, tag=f"outT{bc}", name=f"outT{bc}")
                       for bc in range(NBC)]

            # ---------------- build V[(q<=15,iz), b] (bf16, DVE only) -------
            # V = relu(min(L, R)) except the extrapolation rows: q=1 keeps L
            # un-relu'd, q=15 keeps R un-relu'd (each one fused stt).
            V_sb = []
            for kci in range(NKC):
                cs = slice(kci * B, (kci + 1) * B)
                vt = per.tile([128, B], BF16, tag=f"V{kci}", name=f"V{kci}")
                mn = per.tile([128, B], BF16, tag=f"vmn{kci}", name=f"vmn{kci}")
                nc.vector.tensor_tensor(mn[:], zvLt[:, cs], zvRt[:, cs], ALU.min)
                nc.vector.tensor_scalar_max(vt[:], mn[:], 0.0)
                if kci == 0:
                    # q=1 rows 32:64: V = min(L, relu(R))
                    nc.vector.scalar_tensor_tensor(
                        out=vt[32:64, :], in0=zvRt[32:64, cs], scalar=0.0,
                        in1=zvLt[32:64, cs], op0=ALU.max, op1=ALU.min)
                if kci == 3:
                    # q=15 rows 96:128: V = min(relu(L), R)
                    nc.vector.scalar_tensor_tensor(
                        out=vt[96:128, :], in0=zvLt[96:128, cs], scalar=0.0,
                        in1=zvRt[96:128, cs], op0=ALU.max, op1=ALU.min)
                V_sb.append(vt)

            # ---------------- build U[b, (p<=15,ix)] (bf16, DVE only) -------
            # uLR packs [uL | uR] per batch chunk; same relu(min) structure,
            # p=1 keeps L un-relu'd, p=15 keeps R un-relu'd.
            U_sb = []
            for bc in range(NBC):
                cl = slice(bc * 2 * M, bc * 2 * M + M)
                cr = slice(bc * 2 * M + M, (bc + 1) * 2 * M)
                ut = per.tile([128, M], BF16, tag=f"U{bc}", name=f"U{bc}")
                mu = per.tile([128, M], BF16, tag=f"umn{bc}", name=f"umn{bc}")
                nc.vector.tensor_tensor(mu[:], uLRt[:, cl], uLRt[:, cr], ALU.min)
                nc.vector.tensor_scalar_max(ut[:], mu[:], 0.0)
                l0 = bc * 2 * M
                nc.vector.scalar_tensor_tensor(
                    out=ut[:, 32:64], in0=uLRt[:, l0 + M + 32:l0 + M + 64],
                    scalar=0.0, in1=uLRt[:, l0 + 32:l0 + 64],
                    op0=ALU.max, op1=ALU.min)
                nc.vector.scalar_tensor_tensor(
                    out=ut[:, 480:512], in0=uLRt[:, l0 + 480:l0 + 512],
                    scalar=0.0, in1=uLRt[:, l0 + M + 480:l0 + M + 512],
                    op0=ALU.max, op1=ALU.min)
                U_sb.append(ut)

            # ---------------- main loop ----------------
            # kc-major within each o: 4 batch chains interleaved across 4
            # PSUM banks so consecutive matmuls pipeline.
            for o in range(OSH):
                psts = [ps.tile([128, M], F32, tag="ps", name=f"ps{o}_{bc}")
                        for bc in range(NBC)]
                for kci in range(NKC):
                    st = kci == 0
                    sp = kci == NKC - 1
                    for bc in range(NBC):
                        bs = slice(bc * 128, (bc + 1) * 128)
                        nc.tensor.matmul(psts[bc][:], V_sb[kci][:, bs],
                                         G_sb[o][:, kci * M:(kci + 1) * M],
                                         start=st, stop=sp)
                for bc in range(NBC):
                    jk = junk.tile([128, M], BF16, tag="junk", name=f"jk{o}_{bc}")
                    if o == OSH - 1:
                        # last column: reduce straight out of PSUM on DVE to
                        # shorten the drain tail (no ACT copy latency)
                        nc.vector.scalar_tensor_tensor(
                            out=jk[:], in0=psts[bc][:], scalar=1.0,
                            in1=U_sb[bc][:], op0=ALU.mult, op1=ALU.mult,
                            accum_out=outT_sb[bc][:, o:o + 1])
                    else:
                        cp = scb.tile([128, M], BF16, tag="scb", name=f"cp{o}_{bc}")
                        nc.scalar.activation(cp[:], psts[bc][:], AF.Copy)
                        nc.vector.scalar_tensor_tensor(
                            out=jk[:], in0=cp[:], scalar=1.0, in1=U_sb[bc][:],
                            op0=ALU.mult, op1=ALU.mult,
                            accum_out=outT_sb[bc][:, o:o + 1])

            # ---------------- store ----------------
            for bc in range(NBC):
                nc.sync.dma_start(out_d[bc * 128:(bc + 1) * 128, :], outT_sb[bc][:])

    nc.finalize()
    return nc


def _hat_arms(vals, bd, il):
    """L/R hat arms per (corner, elem, b): L_q = (v-bd[q-1])*il[q-1],
    R_q = (bd[q+1]-v)*il[q]; +-BIG where the arm does not exist."""
    q = np.arange(NQ)
    aL = np.where(q >= 1, il[np.clip(q - 1, 0, NG - 1)], 0.0).astype(np.float32)
    bL = np.where(q >= 1, -bd[np.clip(q - 1, 0, NQ - 1)] * il[np.clip(q - 1, 0, NG - 1)], BIG).astype(np.float32)
    aR = np.where(q <= NG - 1, -il[np.clip(q, 0, NG - 1)], 0.0).astype(np.float32)
    bR = np.where(q <= NG - 1, bd[np.clip(q + 1, 0, NQ - 1)] * il[np.clip(q, 0, NG - 1)], BIG).astype(np.float32)
    L = aL[:, None, None] * vals[None, :, :] + bL[:, None, None]
    R = aR[:, None, None] * vals[None, :, :] + bR[:, None, None]
    return L, R


def _hat_weights(vals, bd, il):
    """Exact fp32 hat weights [(q<=15, elem), b] (mirrors the device build)."""
    L, R = _hat_arms(vals, bd, il)
    W = np.maximum(np.minimum(L, R), 0.0)
    W[1] = np.minimum(L[1], np.maximum(R[1], 0.0))     # q=1: L un-relu'd
    W[15] = np.minimum(np.maximum(L[15], 0.0), R[15])  # q=15: R un-relu'd
    n = vals.shape[0]
    return W[0:NG].reshape(NG * n, -1)


def _host_prep(x, z, func_parameter, borders, il):
    import ml_dtypes
    x = np.asarray(x, np.float32)
    z = np.asarray(z, np.float32)
    F = np.asarray(func_parameter, np.float32)
    bd = np.asarray(borders, np.float32)
    il = np.asarray(il, np.float32)
    bf = ml_dtypes.bfloat16

    # G_all[o, q*32+iz, p*32+ix] = F[p,q,o,ix,iz]
    K = NQ * IZ
    G_all = np.ascontiguousarray(F.transpose(2, 1, 4, 0, 3)).reshape(OUT, K, K)
    # x-side fold (drop p=16 cols), then z-side fold (drop q=16 rows)
    Ghat = G_all[:, :, 0:M] - np.tile(G_all[:, :, M:K], (1, 1, NG))
    gr = G_all[:, :, M:K].sum(axis=2)                         # [64, 544]
    Gh5 = Ghat.reshape(OUT, NQ, IZ, M)
    Ghh = np.ascontiguousarray(
        (Gh5[:, 0:NG] - Gh5[:, NG:NQ]).reshape(OUT, KF, M))   # [64, 512, 512]
    cbt = Gh5[:, NG].sum(axis=1)                              # [64, 512]
    gr5 = gr.reshape(OUT, NQ, IZ)
    gr2 = (gr5[:, 0:NG] - gr5[:, NG:NQ]).reshape(OUT, KF)     # [64, 512]
    c0 = gr5[:, NG].sum(axis=1)                               # [64]

    # exact fp32 side correction, added during output assembly
    V16 = _hat_weights(z, bd, il)                             # [512, 512]
    U16 = _hat_weights(x, bd, il)                             # [512, 512]
    side = cbt @ U16 + gr2 @ V16 + c0[:, None]                # [64, 512]

    # device arm tensors (bf16)
    zL, zR = _hat_arms(z, bd, il)
    zL = zL[0:NG].reshape(KF, B)
    zR = zR[0:NG].reshape(KF, B)
    zvl = np.ascontiguousarray(
        zL.reshape(NKC, 128, B).transpose(1, 0, 2).reshape(128, NKC * B))
    zvr = np.ascontiguousarray(
        zR.reshape(NKC, 128, B).transpose(1, 0, 2).reshape(128, NKC * B))

    xL, xR = _hat_arms(x, bd, il)
    xLb = xL[0:NG].reshape(M, B).T                            # [512b, 512m]
    xRb = xR[0:NG].reshape(M, B).T
    ulr = np.empty((128, 8 * M), np.float32)
    for bc in range(NBC):
        ulr[:, bc * 2 * M:bc * 2 * M + M] = xLb[bc * 128:(bc + 1) * 128]
        ulr[:, bc * 2 * M + M:(bc + 1) * 2 * M] = xRb[bc * 128:(bc + 1) * 128]

    gmain_all = []
    for c in range(NCORES):
        Go = Ghh[c * OSH:(c + 1) * OSH]                       # [8, 512, 512]
        gmain = np.ascontiguousarray(
            Go.reshape(OSH, NKC, 128, M).transpose(0, 2, 1, 3)
            .reshape(OSH, 128, NKC * M)).astype(bf)
        gmain_all.append(gmain)
    return (gmain_all, zvl.astype(bf), zvr.astype(bf), ulr.astype(bf),
            side, bd, il)


def kernel(x, z, func_parameter, borders, inverse_chunk_lengths, _trace=False):
    gmain_all, zvl, zvr, ulr, side, bd, il = _host_prep(
        x, z, func_parameter, borders, inverse_chunk_lengths)

    key = (bd.tobytes(), il.tobytes())
    if key not in _NC_CACHE:
        _NC_CACHE[key] = _build_nc(bd, il)
    nc = _NC_CACHE[key]

    in_maps = []
    for c in range(NCORES):
        in_maps.append({
            "gmain": gmain_all[c],
            "zvl": zvl,
            "zvr": zvr,
            "ulr": ulr,
        })

    res = run_bass_kernel_spmd(nc, in_maps, core_ids=list(range(NCORES)),
                               trace=_trace)
    parts = []
    for c in range(NCORES):
        r = res.results[c]
        parts.append(r["out"].T.astype(np.float32) + side[c * OSH:(c + 1) * OSH])
    out = np.ascontiguousarray(np.concatenate(parts, axis=0).astype(np.float32))
    if _trace:
        return out, res
    return out


# revision 26
# speedup vs baseline: 1.0413x; 1.0413x over previous
"""Trainium2 Bass kernel for nn_BasisFunction2D (2-D basis-function embedding lookup).

Reformulation: data-dependent bilinear interpolation over a 16x16 grid of
per-(ix,iz) tables expressed as dense hat-function interpolation matrices

    V[(q,iz), b] = hat_q(z[iz,b])      (z-side weights, 2 nonzeros per column)
    U[(p,ix), b] = hat_p(x[ix,b])      (x-side weights)

with partition-of-unity folds on BOTH axes (sum_q hat_q = sum_p hat_p = 1,
exact even in the linear-extrapolation tails), which shrink the contraction
to K=512 (4 full PE chunks, no ragged tail) and the free dim to M=512 (one
PSUM bank per chain):

    out[o,b] = sum_m C_o[b,m] * U[m,b]  +  side[o,b]
    C_o[b,m] = sum_{k in 512} V[k,b] * Ghh_o[k,m]         (PE, bf16, N=512)
    Ghh = doubly-folded table;  side = cbt_o @ U + gr2_o @ V + c0_o is a
    rank-small exact correction (0.35% of the FLOPs) evaluated on the host
    in fp32 and added during output assembly.

Within an output column o the four batch chains are emitted kc-major
(interleaved), so consecutive matmuls target different PSUM banks and
pipeline through the PE reorder window at the N=512 streaming rate.

The hat arms L/R are host-precomputed affine maps of z/x (bf16); the device
builds V = relu(min(L,R)) on the vector engine (2 ops/chunk + 1-op stt fixes
for the extrapolation rows).  Stage 2 drains each chain with an ACT
PSUM->SBUF bf16 copy + DVE fused multiply-reduce against U (the last column
reduces straight out of PSUM to shorten the tail).
"""

import numpy as np

import concourse.bass as bass
import concourse.bacc as bacc_mod
import concourse.tile as tile
from concourse import mybir
from concourse.bass_utils import run_bass_kernel_spmd

F32 = mybir.dt.float32
BF16 = mybir.dt.bfloat16
AF = mybir.ActivationFunctionType
ALU = mybir.AluOpType

NCORES = 8
NG = 16            # grid bins
NQ = 17            # grid corners per axis
IX = 32
IZ = 32
OUT = 64
B = 512
OSH = OUT // NCORES          # outputs per core = 8
KF = NG * IZ                 # 512 folded contraction rows (q<=15, iz)
M = NG * IX                  # 512 folded free cols (p<=15, ix)
BIG = 1e30
NBC = B // 128               # 4 batch chunks
NKC = 4                      # contraction chunks of 128
NWARM = 16                   # PE warmup matmuls (p-state ramp + DMA cover)

_NC_CACHE = {}


def _build_nc(bd, il):
    """Build the single-core Bass/Tile program (identical across cores)."""
    nc = bacc_mod.Bacc(None, target_bir_lowering=False)
    gmain_d = nc.dram_tensor("gmain", [OSH, 128, NKC * M], BF16, kind="ExternalInput")
    zvl_d = nc.dram_tensor("zvl", [128, NKC * B], BF16, kind="ExternalInput")
    zvr_d = nc.dram_tensor("zvr", [128, NKC * B], BF16, kind="ExternalInput")
    ulr_d = nc.dram_tensor("ulr", [128, 8 * M], BF16, kind="ExternalInput")
    out_d = nc.dram_tensor("out", [B, OSH], F32, kind="ExternalOutput")

    with tile.TileContext(nc) as tc:
        with (
            tc.tile_pool(name="per", bufs=1) as per,       # persistent tiles
            tc.tile_pool(name="scb", bufs=4) as scb,       # stage2 ACT copies
            tc.tile_pool(name="junk", bufs=2) as junk,     # stt mandatory outs
            tc.tile_pool(name="ps", bufs=7, space="PSUM") as ps,
            tc.tile_pool(name="ps2", bufs=1, space="PSUM") as ps2,
        ):
            # ---------------- PE warmup ----------------
            # Dependency-free dummy matmuls keep the PE busy through the DMA
            # and V build phase so the p-state ramps before the real chains.
            wt = per.tile([128, B], BF16, tag="warm", name="wt")
            nc.vector.memset(wt[:], 0.0)
            wps = ps2.tile([128, B], F32, tag="w", name="wps")
            for _ in range(NWARM):
                nc.tensor.matmul(wps[:], wt[:, 0:128], wt[:], start=True, stop=True)

            # ---------------- input loads ----------------
            # V arm chunks split across the sync and scalar trigger queues
            # (each trigger costs ~650ns of queue time); the gpsimd queue
            # (slow: framework drains) gets the early o=0 G tile.
            zvLt = per.tile([128, NKC * B], BF16, tag="zvL", name="zvLt")
            zvRt = per.tile([128, NKC * B], BF16, tag="zvR", name="zvRt")
            uLRt = per.tile([128, 8 * M], BF16, tag="uLR", name="uLRt")
            G_sb = [per.tile([128, NKC * M], BF16, tag=f"G{o}", name=f"G{o}")
                    for o in range(OSH)]

            # the V arms get the early fabric bandwidth to themselves (plus
            # G0); the remaining G tiles trail behind uLR on the sync queue.
            nc.gpsimd.dma_start(G_sb[0][:], gmain_d[0])
            for kci in range(NKC):
                cs = slice(kci * B, (kci + 1) * B)
                nc.sync.dma_start(zvLt[:, cs], zvl_d[:, cs])
                nc.scalar.dma_start(zvRt[:, cs], zvr_d[:, cs])
            for bc in range(NBC):
                ub = slice(bc * 2 * M, (bc + 1) * 2 * M)
                nc.sync.dma_start(uLRt[:, ub], ulr_d[:, ub])
            for o in range(1, OSH):
                nc.sync.dma_start(G_sb[o][:], gmain_d[o])

            outT_sb = [per.tile([128, OSH], F32, tag=f"outT{bc}", name=f"outT{bc}")
                       for bc in range(NBC)]

            # ---------------- build V[(q<=15,iz), b] (bf16, DVE only) -------
            # V = relu(min(L, R)) except the extrapolation rows: q=1 keeps L
            # un-relu'd, q=15 keeps R un-relu'd (each one fused stt).
            V_sb = []
            for kci in range(NKC):
                cs = slice(kci * B, (kci + 1) * B)
                vt = per.tile([128, B], BF16, tag=f"V{kci}", name=f"V{kci}")
                mn = per.tile([128, B], BF16, tag=f"vmn{kci}", name=f"vmn{kci}")
                nc.vector.tensor_tensor(mn[:], zvLt[:, cs], zvRt[:, cs], ALU.min)
                nc.vector.tensor_scalar_max(vt[:], mn[:], 0.0)
                if kci == 0:
                    # q=1 rows 32:64: V = min(L, relu(R))
                    nc.vector.scalar_tensor_tensor(
                        out=vt[32:64, :], in0=zvRt[32:64, cs], scalar=0.0,
                        in1=zvLt[32:64, cs], op0=ALU.max, op1=ALU.min)
                if kci == 3:
                    # q=15 rows 96:128: V = min(relu(L), R)
                    nc.vector.scalar_tensor_tensor(
                        out=vt[96:128, :], in0=zvLt[96:128, cs], scalar=0.0,
                        in1=zvRt[96:128, cs], op0=ALU.max, op1=ALU.min)
                V_sb.append(vt)

            # ---------------- build U[b, (p<=15,ix)] (bf16, DVE only) -------
            # uLR packs [uL | uR] per batch chunk; same relu(min) structure,
            # p=1 keeps L un-relu'd, p=15 keeps R un-relu'd.
            U_sb = []
            for bc in range(NBC):
                cl = slice(bc * 2 * M, bc * 2 * M + M)
                cr = slice(bc * 2 * M + M, (bc + 1) * 2 * M)
                ut = per.tile([128, M], BF16, tag=f"U{bc}", name=f"U{bc}")
                mu = per.tile([128, M], BF16, tag=f"umn{bc}", name=f"umn{bc}")
                nc.vector.tensor_tensor(mu[:], uLRt[:, cl], uLRt[:, cr], ALU.min)
                nc.vector.tensor_scalar_max(ut[:], mu[:], 0.0)
                l0 = bc * 2 * M
                nc.vector.scalar_tensor_tensor(
                    out=ut[:, 32:64], in0=uLRt[:, l0 + M + 32:l0 + M + 64],
                    scalar=0.0, in1=uLRt[:, l0 + 32:l0 + 64],
                    op0=ALU.max, op1=ALU.min)
                nc.vector.scalar_tensor_tensor(
                    out=ut[:, 480:512], in0=uLRt[:, l0 + 480:l0 + 512],
                    scalar=0.0, in1=uLRt[:, l0 + M + 480:l0 + M + 512],
                    op0=ALU.max, op1=ALU.min)
                U_sb.append(ut)

            # ---------------- main loop ----------------
            # kc-major within each o: 4 batch chains interleaved across 4
            # PSUM banks so consecutive matmuls pipeline.
            for o in range(OSH):
                psts = [ps.tile([128, M], F32, tag="ps", name=f"ps{o}_{bc}")
                        for bc in range(NBC)]
                for kci in range(NKC):
                    st = kci == 0
                    sp = kci == NKC - 1
                    for bc in range(NBC):
                        bs = slice(bc * 128, (bc + 1) * 128)
                        nc.tensor.matmul(psts[bc][:], V_sb[kci][:, bs],
                                         G_sb[o][:, kci * M:(kci + 1) * M],
                                         start=st, stop=sp)
                for bc in range(NBC):
                    jk = junk.tile([128, M], BF16, tag="junk", name=f"jk{o}_{bc}")
                    if o == OSH - 1:
                        # last column: reduce straight out of PSUM on DVE to
                        # shorten the drain tail (no ACT copy latency)
                        nc.vector.scalar_tensor_tensor(
                            out=jk[:], in0=psts[bc][:], scalar=1.0,
                            in1=U_sb[bc][:], op0=ALU.mult, op1=ALU.mult,
                            accum_out=outT_sb[bc][:, o:o + 1])
                    else:
                        cp = scb.tile([128, M], BF16, tag="scb", name=f"cp{o}_{bc}")
                        nc.scalar.activation(cp[:], psts[bc][:], AF.Copy)
                        nc.vector.scalar_tensor_tensor(
                            out=jk[:], in0=cp[:], scalar=1.0, in1=U_sb[bc][:],
                            op0=ALU.mult, op1=ALU.mult,
                            accum_out=outT_sb[bc][:, o:o + 1])

            # ---------------- store ----------------
            for bc in range(NBC):
                nc.sync.dma_start(out_d[bc * 128:(bc + 1) * 128, :], outT_sb[bc][:])

    nc.finalize()
    return nc


def _hat_arms(vals, bd, il):
    """L/R hat arms per (corner, elem, b): L_q = (v-bd[q-1])*il[q-1],
    R_q = (bd[q+1]-v)*il[q]; +-BIG where the arm does not exist."""
    q = np.arange(NQ)
    aL = np.where(q >= 1, il[np.clip(q - 1, 0, NG - 1)], 0.0).astype(np.float32)
    bL = np.where(q >= 1, -bd[np.clip(q - 1, 0, NQ - 1)] * il[np.clip(q - 1, 0, NG - 1)], BIG).astype(np.float32)
    aR = np.where(q <= NG - 1, -il[np.clip(q, 0, NG - 1)], 0.0).astype(np.float32)
    bR = np.where(q <= NG - 1, bd[np.clip(q + 1, 0, NQ - 1)] * il[np.clip(q, 0, NG - 1)], BIG).astype(np.float32)
    L = aL[:, None, None] * vals[None, :, :] + bL[:, None, None]
    R = aR[:, None, None] * vals[None, :, :] + bR[:, None, None]
    return L, R


def _hat_weights(vals, bd, il):
    """Exact fp32 hat weights [(q<=15, elem), b] (mirrors the device build)."""
    L, R = _hat_arms(vals, bd, il)
    W = np.maximum(np.minimum(L, R), 0.0)
    W[1] = np.minimum(L[1], np.maximum(R[1], 0.0))     # q=1: L un-relu'd
    W[15] = np.minimum(np.maximum(L[15], 0.0), R[15])  # q=15: R un-relu'd
    n = vals.shape[0]
    return W[0:NG].reshape(NG * n, -1)


def _host_prep(x, z, func_parameter, borders, il):
    import ml_dtypes
    x = np.asarray(x, np.float32)
    z = np.asarray(z, np.float32)
    F = np.asarray(func_parameter, np.float32)
    bd = np.asarray(borders, np.float32)
    il = np.asarray(il, np.float32)
    bf = ml_dtypes.bfloat16

    # G_all[o, q*32+iz, p*32+ix] = F[p,q,o,ix,iz]
    K = NQ * IZ
    G_all = np.ascontiguousarray(F.transpose(2, 1, 4, 0, 3)).reshape(OUT, K, K)
    # x-side fold (drop p=16 cols), then z-side fold (drop q=16 rows)
    Ghat = G_all[:, :, 0:M] - np.tile(G_all[:, :, M:K], (1, 1, NG))
    gr = G_all[:, :, M:K].sum(axis=2)                         # [64, 544]
    Gh5 = Ghat.reshape(OUT, NQ, IZ, M)
    Ghh = np.ascontiguousarray(
        (Gh5[:, 0:NG] - Gh5[:, NG:NQ]).reshape(OUT, KF, M))   # [64, 512, 512]
    cbt = Gh5[:, NG].sum(axis=1)                              # [64, 512]
    gr5 = gr.reshape(OUT, NQ, IZ)
    gr2 = (gr5[:, 0:NG] - gr5[:, NG:NQ]).reshape(OUT, KF)     # [64, 512]
    c0 = gr5[:, NG].sum(axis=1)                               # [64]

    # exact fp32 side correction, added during output assembly
    V16 = _hat_weights(z, bd, il)                             # [512, 512]
    U16 = _hat_weights(x, bd, il)                             # [512, 512]
    side = cbt @ U16 + gr2 @ V16 + c0[:, None]                # [64, 512]

    # device arm tensors (bf16)
    zL, zR = _hat_arms(z, bd, il)
    zL = zL[0:NG].reshape(KF, B)
    zR = zR[0:NG].reshape(KF, B)
    zvl = np.ascontiguousarray(
        zL.reshape(NKC, 128, B).transpose(1, 0, 2).reshape(128, NKC * B))
    zvr = np.ascontiguousarray(
        zR.reshape(NKC, 128, B).transpose(1, 0, 2).reshape(128, NKC * B))

    xL, xR = _hat_arms(x, bd, il)
    xLb = xL[0:NG].reshape(M, B).T                            # [512b, 512m]
    xRb = xR[0:NG].reshape(M, B).T
    ulr = np.empty((128, 8 * M), np.float32)
    for bc in range(NBC):
        ulr[:, bc * 2 * M:bc * 2 * M + M] = xLb[bc * 128:(bc + 1) * 128]
        ulr[:, bc * 2 * M + M:(bc + 1) * 2 * M] = xRb[bc * 128:(bc + 1) * 128]

    gmain_all = []
    for c in range(NCORES):
        Go = Ghh[c * OSH:(c + 1) * OSH]                       # [8, 512, 512]
        gmain = np.ascontiguousarray(
            Go.reshape(OSH, NKC, 128, M).transpose(0, 2, 1, 3)
            .reshape(OSH, 128, NKC * M)).astype(bf)
        gmain_all.append(gmain)
    return (gmain_all, zvl.astype(bf), zvr.astype(bf), ulr.astype(bf),
            side, bd, il)


def kernel(x, z, func_parameter, borders, inverse_chunk_lengths, _trace=False):
    gmain_all, zvl, zvr, ulr, side, bd, il = _host_prep(
        x, z, func_parameter, borders, inverse_chunk_lengths)

    key = (bd.tobytes(), il.tobytes())
    if key not in _NC_CACHE:
        _NC_CACHE[key] = _build_nc(bd, il)
    nc = _NC_CACHE[key]

    in_maps = []
    for c in range(NCORES):
        in_maps.append({
            "gmain": gmain_all[c],
            "zvl": zvl,
            "zvr": zvr,
            "ulr": ulr,
        })

    res = run_bass_kernel_spmd(nc, in_maps, core_ids=list(range(NCORES)),
                               trace=_trace)
    parts = []
    for c in range(NCORES):
        r = res.results[c]
        parts.append(r["out"].T.astype(np.float32) + side[c * OSH:(c + 1) * OSH])
    out = np.ascontiguousarray(np.concatenate(parts, axis=0).astype(np.float32))
    if _trace:
        return out, res
    return out


# revision 27
# speedup vs baseline: 1.0687x; 1.0263x over previous
"""Trainium2 Bass kernel for nn_BasisFunction2D (2-D basis-function embedding lookup).

Reformulation: data-dependent bilinear interpolation over a 16x16 grid of
per-(ix,iz) tables expressed as dense hat-function interpolation matrices

    V[(q,iz), b] = hat_q(z[iz,b])      (z-side weights, 2 nonzeros per column)
    U[(p,ix), b] = hat_p(x[ix,b])      (x-side weights)

with partition-of-unity folds on BOTH axes (sum_q hat_q = sum_p hat_p = 1,
exact even in the linear-extrapolation tails), which shrink the contraction
to K=512 (4 full PE chunks, no ragged tail) and the free dim to M=512 (one
PSUM bank per chain):

    out[o,b] = sum_m C_o[b,m] * U[m,b]  +  side[o,b]
    C_o[b,m] = sum_{k in 512} V[k,b] * Ghh_o[k,m]         (PE, bf16, N=512)
    Ghh = doubly-folded table;  side = cbt_o @ U + gr2_o @ V + c0_o is a
    rank-small exact correction (0.35% of the FLOPs) evaluated on the host
    in fp32 and added during output assembly.

Within an output column o the four batch chains are emitted kc-major
(interleaved), so consecutive matmuls target different PSUM banks and
pipeline through the PE reorder window at the N=512 streaming rate.

The hat arms L/R are host-precomputed affine maps of z/x (bf16); the device
builds V = relu(min(L,R)) on the vector engine (2 ops/chunk + 1-op stt fixes
for the extrapolation rows).  Stage 2 drains each chain with an ACT
PSUM->SBUF bf16 copy + DVE fused multiply-reduce against U (the last column
reduces straight out of PSUM to shorten the tail).
"""

import numpy as np

import concourse.bass as bass
import concourse.bacc as bacc_mod
import concourse.tile as tile
from concourse import mybir
from concourse.bass_utils import run_bass_kernel_spmd

F32 = mybir.dt.float32
BF16 = mybir.dt.bfloat16
AF = mybir.ActivationFunctionType
ALU = mybir.AluOpType

NCORES = 8
NG = 16            # grid bins
NQ = 17            # grid corners per axis
IX = 32
IZ = 32
OUT = 64
B = 512
OSH = OUT // NCORES          # outputs per core = 8
KF = NG * IZ                 # 512 folded contraction rows (q<=15, iz)
M = NG * IX                  # 512 folded free cols (p<=15, ix)
BIG = 1e30
NBC = B // 128               # 4 batch chunks
NKC = 4                      # contraction chunks of 128
NWARM = 10                   # PE warmup matmuls (p-state ramp + DMA cover)

_NC_CACHE = {}


def _build_nc(bd, il):
    """Build the single-core Bass/Tile program (identical across cores)."""
    nc = bacc_mod.Bacc(None, target_bir_lowering=False)
    gmain_d = nc.dram_tensor("gmain", [OSH, 128, NKC * M], BF16, kind="ExternalInput")
    zvl_d = nc.dram_tensor("zvl", [128, NKC * B], BF16, kind="ExternalInput")
    zvr_d = nc.dram_tensor("zvr", [128, NKC * B], BF16, kind="ExternalInput")
    ulr_d = nc.dram_tensor("ulr", [128, 8 * M], BF16, kind="ExternalInput")
    out_d = nc.dram_tensor("out", [B, OSH], F32, kind="ExternalOutput")

    with tile.TileContext(nc) as tc:
        with (
            tc.tile_pool(name="per", bufs=1) as per,       # persistent tiles
            tc.tile_pool(name="scb", bufs=4) as scb,       # stage2 ACT copies
            tc.tile_pool(name="junk", bufs=2) as junk,     # stt mandatory outs
            tc.tile_pool(name="ps", bufs=7, space="PSUM") as ps,
            tc.tile_pool(name="ps2", bufs=1, space="PSUM") as ps2,
        ):
            # ---------------- PE warmup ----------------
            # Dependency-free dummy matmuls keep the PE busy through the DMA
            # and V build phase so the p-state ramps before the real chains.
            wt = per.tile([128, B], BF16, tag="warm", name="wt")
            nc.vector.memset(wt[:], 0.0)
            wps = ps2.tile([128, B], F32, tag="w", name="wps")
            for _ in range(NWARM):
                nc.tensor.matmul(wps[:], wt[:, 0:128], wt[:], start=True, stop=True)

            # ---------------- input loads ----------------
            # V arm chunks split across the sync and scalar trigger queues
            # (each trigger costs ~650ns of queue time); the gpsimd queue
            # (slow: framework drains) gets the early o=0 G tile.
            zvLt = per.tile([128, NKC * B], BF16, tag="zvL", name="zvLt")
            zvRt = per.tile([128, NKC * B], BF16, tag="zvR", name="zvRt")
            uLRt = per.tile([128, 8 * M], BF16, tag="uLR", name="uLRt")
            G_sb = [per.tile([128, NKC * M], BF16, tag=f"G{o}", name=f"G{o}")
                    for o in range(OSH)]

            # the V arms get the early fabric bandwidth to themselves (plus
            # G0); the remaining G tiles trail behind uLR on the sync queue.
            nc.gpsimd.dma_start(G_sb[0][:], gmain_d[0])
            for kci in range(NKC):
                cs = slice(kci * B, (kci + 1) * B)
                nc.sync.dma_start(zvLt[:, cs], zvl_d[:, cs])
                nc.scalar.dma_start(zvRt[:, cs], zvr_d[:, cs])
            for bc in range(NBC):
                ub = slice(bc * 2 * M, (bc + 1) * 2 * M)
                nc.sync.dma_start(uLRt[:, ub], ulr_d[:, ub])
            for o in range(1, OSH):
                nc.sync.dma_start(G_sb[o][:], gmain_d[o])

            outT_sb = [per.tile([128, OSH], F32, tag=f"outT{bc}", name=f"outT{bc}")
                       for bc in range(NBC)]

            # ---------------- build V[(q<=15,iz), b] (bf16, DVE only) -------
            # V = relu(min(L, R)) except the extrapolation rows: q=1 keeps L
            # un-relu'd, q=15 keeps R un-relu'd (each one fused stt).
            V_sb = []
            for kci in range(NKC):
                cs = slice(kci * B, (kci + 1) * B)
                vt = per.tile([128, B], BF16, tag=f"V{kci}", name=f"V{kci}")
                mn = per.tile([128, B], BF16, tag=f"vmn{kci}", name=f"vmn{kci}")
                nc.vector.tensor_tensor(mn[:], zvLt[:, cs], zvRt[:, cs], ALU.min)
                nc.vector.tensor_scalar_max(vt[:], mn[:], 0.0)
                if kci == 0:
                    # q=1 rows 32:64: V = min(L, relu(R))
                    nc.vector.scalar_tensor_tensor(
                        out=vt[32:64, :], in0=zvRt[32:64, cs], scalar=0.0,
                        in1=zvLt[32:64, cs], op0=ALU.max, op1=ALU.min)
                if kci == 3:
                    # q=15 rows 96:128: V = min(relu(L), R)
                    nc.vector.scalar_tensor_tensor(
                        out=vt[96:128, :], in0=zvLt[96:128, cs], scalar=0.0,
                        in1=zvRt[96:128, cs], op0=ALU.max, op1=ALU.min)
                V_sb.append(vt)

            # ---------------- build U[b, (p<=15,ix)] (bf16, DVE only) -------
            # uLR packs [uL | uR] per batch chunk; same relu(min) structure,
            # p=1 keeps L un-relu'd, p=15 keeps R un-relu'd.
            U_sb = []
            for bc in range(NBC):
                cl = slice(bc * 2 * M, bc * 2 * M + M)
                cr = slice(bc * 2 * M + M, (bc + 1) * 2 * M)
                ut = per.tile([128, M], BF16, tag=f"U{bc}", name=f"U{bc}")
                mu = per.tile([128, M], BF16, tag=f"umn{bc}", name=f"umn{bc}")
                nc.vector.tensor_tensor(mu[:], uLRt[:, cl], uLRt[:, cr], ALU.min)
                nc.vector.tensor_scalar_max(ut[:], mu[:], 0.0)
                l0 = bc * 2 * M
                nc.vector.scalar_tensor_tensor(
                    out=ut[:, 32:64], in0=uLRt[:, l0 + M + 32:l0 + M + 64],
                    scalar=0.0, in1=uLRt[:, l0 + 32:l0 + 64],
                    op0=ALU.max, op1=ALU.min)
                nc.vector.scalar_tensor_tensor(
                    out=ut[:, 480:512], in0=uLRt[:, l0 + 480:l0 + 512],
                    scalar=0.0, in1=uLRt[:, l0 + M + 480:l0 + M + 512],
                    op0=ALU.max, op1=ALU.min)
                U_sb.append(ut)

            # ---------------- main loop ----------------
            # kc-major within each o: 4 batch chains interleaved across 4
            # PSUM banks so consecutive matmuls pipeline.
            for o in range(OSH):
                psts = [ps.tile([128, M], F32, tag="ps", name=f"ps{o}_{bc}")
                        for bc in range(NBC)]
                for kci in range(NKC):
                    st = kci == 0
                    sp = kci == NKC - 1
                    for bc in range(NBC):
                        bs = slice(bc * 128, (bc + 1) * 128)
                        nc.tensor.matmul(psts[bc][:], V_sb[kci][:, bs],
                                         G_sb[o][:, kci * M:(kci + 1) * M],
                                         start=st, stop=sp)
                for bc in range(NBC):
                    jk = junk.tile([128, M], BF16, tag="junk", name=f"jk{o}_{bc}")
                    if o == OSH - 1:
                        # last column: reduce straight out of PSUM on DVE to
                        # shorten the drain tail (no ACT copy latency)
                        nc.vector.scalar_tensor_tensor(
                            out=jk[:], in0=psts[bc][:], scalar=1.0,
                            in1=U_sb[bc][:], op0=ALU.mult, op1=ALU.mult,
                            accum_out=outT_sb[bc][:, o:o + 1])
                    else:
                        cp = scb.tile([128, M], BF16, tag="scb", name=f"cp{o}_{bc}")
                        nc.scalar.activation(cp[:], psts[bc][:], AF.Copy)
                        nc.vector.scalar_tensor_tensor(
                            out=jk[:], in0=cp[:], scalar=1.0, in1=U_sb[bc][:],
                            op0=ALU.mult, op1=ALU.mult,
                            accum_out=outT_sb[bc][:, o:o + 1])

            # ---------------- store ----------------
            for bc in range(NBC):
                nc.sync.dma_start(out_d[bc * 128:(bc + 1) * 128, :], outT_sb[bc][:])

    nc.finalize()
    return nc


def _hat_arms(vals, bd, il):
    """L/R hat arms per (corner, elem, b): L_q = (v-bd[q-1])*il[q-1],
    R_q = (bd[q+1]-v)*il[q]; +-BIG where the arm does not exist."""
    q = np.arange(NQ)
    aL = np.where(q >= 1, il[np.clip(q - 1, 0, NG - 1)], 0.0).astype(np.float32)
    bL = np.where(q >= 1, -bd[np.clip(q - 1, 0, NQ - 1)] * il[np.clip(q - 1, 0, NG - 1)], BIG).astype(np.float32)
    aR = np.where(q <= NG - 1, -il[np.clip(q, 0, NG - 1)], 0.0).astype(np.float32)
    bR = np.where(q <= NG - 1, bd[np.clip(q + 1, 0, NQ - 1)] * il[np.clip(q, 0, NG - 1)], BIG).astype(np.float32)
    L = aL[:, None, None] * vals[None, :, :] + bL[:, None, None]
    R = aR[:, None, None] * vals[None, :, :] + bR[:, None, None]
    return L, R


def _hat_weights(vals, bd, il):
    """Exact fp32 hat weights [(q<=15, elem), b] (mirrors the device build)."""
    L, R = _hat_arms(vals, bd, il)
    W = np.maximum(np.minimum(L, R), 0.0)
    W[1] = np.minimum(L[1], np.maximum(R[1], 0.0))     # q=1: L un-relu'd
    W[15] = np.minimum(np.maximum(L[15], 0.0), R[15])  # q=15: R un-relu'd
    n = vals.shape[0]
    return W[0:NG].reshape(NG * n, -1)


def _host_prep(x, z, func_parameter, borders, il):
    import ml_dtypes
    x = np.asarray(x, np.float32)
    z = np.asarray(z, np.float32)
    F = np.asarray(func_parameter, np.float32)
    bd = np.asarray(borders, np.float32)
    il = np.asarray(il, np.float32)
    bf = ml_dtypes.bfloat16

    # G_all[o, q*32+iz, p*32+ix] = F[p,q,o,ix,iz]
    K = NQ * IZ
    G_all = np.ascontiguousarray(F.transpose(2, 1, 4, 0, 3)).reshape(OUT, K, K)
    # x-side fold (drop p=16 cols), then z-side fold (drop q=16 rows)
    Ghat = G_all[:, :, 0:M] - np.tile(G_all[:, :, M:K], (1, 1, NG))
    gr = G_all[:, :, M:K].sum(axis=2)                         # [64, 544]
    Gh5 = Ghat.reshape(OUT, NQ, IZ, M)
    Ghh = np.ascontiguousarray(
        (Gh5[:, 0:NG] - Gh5[:, NG:NQ]).reshape(OUT, KF, M))   # [64, 512, 512]
    cbt = Gh5[:, NG].sum(axis=1)                              # [64, 512]
    gr5 = gr.reshape(OUT, NQ, IZ)
    gr2 = (gr5[:, 0:NG] - gr5[:, NG:NQ]).reshape(OUT, KF)     # [64, 512]
    c0 = gr5[:, NG].sum(axis=1)                               # [64]

    # exact fp32 side correction, added during output assembly
    V16 = _hat_weights(z, bd, il)                             # [512, 512]
    U16 = _hat_weights(x, bd, il)                             # [512, 512]
    side = cbt @ U16 + gr2 @ V16 + c0[:, None]                # [64, 512]

    # device arm tensors (bf16)
    zL, zR = _hat_arms(z, bd, il)
    zL = zL[0:NG].reshape(KF, B)
    zR = zR[0:NG].reshape(KF, B)
    zvl = np.ascontiguousarray(
        zL.reshape(NKC, 128, B).transpose(1, 0, 2).reshape(128, NKC * B))
    zvr = np.ascontiguousarray(
        zR.reshape(NKC, 128, B).transpose(1, 0, 2).reshape(128, NKC * B))

    xL, xR = _hat_arms(x, bd, il)
    xLb = xL[0:NG].reshape(M, B).T                            # [512b, 512m]
    xRb = xR[0:NG].reshape(M, B).T
    ulr = np.empty((128, 8 * M), np.float32)
    for bc in range(NBC):
        ulr[:, bc * 2 * M:bc * 2 * M + M] = xLb[bc * 128:(bc + 1) * 128]
        ulr[:, bc * 2 * M + M:(bc + 1) * 2 * M] = xRb[bc * 128:(bc + 1) * 128]

    gmain_all = []
    for c in range(NCORES):
        Go = Ghh[c * OSH:(c + 1) * OSH]                       # [8, 512, 512]
        gmain = np.ascontiguousarray(
            Go.reshape(OSH, NKC, 128, M).transpose(0, 2, 1, 3)
            .reshape(OSH, 128, NKC * M)).astype(bf)
        gmain_all.append(gmain)
    return (gmain_all, zvl.astype(bf), zvr.astype(bf), ulr.astype(bf),
            side, bd, il)


def kernel(x, z, func_parameter, borders, inverse_chunk_lengths, _trace=False):
    gmain_all, zvl, zvr, ulr, side, bd, il = _host_prep(
        x, z, func_parameter, borders, inverse_chunk_lengths)

    key = (bd.tobytes(), il.tobytes())
    if key not in _NC_CACHE:
        _NC_CACHE[key] = _build_nc(bd, il)
    nc = _NC_CACHE[key]

    in_maps = []
    for c in range(NCORES):
        in_maps.append({
            "gmain": gmain_all[c],
            "zvl": zvl,
            "zvr": zvr,
            "ulr": ulr,
        })

    res = run_bass_kernel_spmd(nc, in_maps, core_ids=list(range(NCORES)),
                               trace=_trace)
    parts = []
    for c in range(NCORES):
        r = res.results[c]
        parts.append(r["out"].T.astype(np.float32) + side[c * OSH:(c + 1) * OSH])
    out = np.ascontiguousarray(np.concatenate(parts, axis=0).astype(np.float32))
    if _trace:
        return out, res
    return out
